# revision 14
# baseline (speedup 1.0000x reference)
"""Trainium2 Bass kernel for nn_BinarizeLayer (histogram_binning).

out[b, f] = 1.0 if (medians[f] > 0) and (inputs[b, f] >= medians[f]) else 0.0

Sharding: pure data-parallel over batch — each of the 8 cores processes a
[1024, 4096] contiguous row shard; the 16 KB medians vector is replicated.

The (median > 0) gate is folded into a per-feature threshold on the host
(thr[f] = medians[f] if medians[f] > 0 else FLT_MAX, a 4096-element
np.where) so the device hot loop is one DVE is_ge compare per element:
inputs are finite floats far below FLT_MAX, so x >= FLT_MAX is never true.

Raw Bass (no Tile): this walrus rejects any instruction carrying more
than one sync-wait, which Tile's generated schedules (and its kernel-tail
drain) violate. With explicit semaphores every wait is its own
single-wait instruction. Pipeline: SP streams the 8 input-chunk loads,
DVE compares each chunk in place as its load lands, ACT streams the
stores behind the compares — loads and stores ride separate HWDGE rings.
"""

import numpy as np

import concourse.bass as bass
import concourse.mybir as mybir
from concourse.bass_utils import run_bass_kernel_spmd

N_CORES = 8
BATCH, FEAT = 8192, 4096
SHARD = BATCH // N_CORES  # 1024 rows per core
P = 128                   # SBUF partitions
ROWG = SHARD // P         # 8 row-groups; DRAM row = p * ROWG + r
BIG = np.float32(3.4e38)  # gate-closed sentinel; x >= BIG never true for inputs

_module = None


def _build_module():
    nc = bass.Bass()
    x = nc.declare_dram_parameter("inputs", [SHARD, FEAT], mybir.dt.float32, isOutput=False)
    thr = nc.declare_dram_parameter("thresholds", [FEAT], mybir.dt.float32, isOutput=False)
    out = nc.declare_dram_parameter("output", [SHARD, FEAT], mybir.dt.float32, isOutput=True)

    # Partition p owns contiguous DRAM rows [p*ROWG, (p+1)*ROWG): each
    # partition's slice of chunk r is one contiguous 16 KB run.
    x3 = x.ap().rearrange("(p r) f -> p r f", p=P)
    out3 = out.ap().rearrange("(p r) f -> p r f", p=P)

    # Chunks: (row-group r, feature offset, width). Small first chunks
    # shorten the ramp (first compute starts sooner); small last chunks
    # shorten the load→compute→store→receipt tail.
    H = FEAT // 2
    Q = FEAT // 4
    chunks = (
        [(0, 0, Q), (0, Q, Q), (0, H, H)]
        + [(r, 0, FEAT) for r in range(1, ROWG - 1)]
        + [(ROWG - 1, 0, H), (ROWG - 1, H, Q), (ROWG - 1, H + Q, Q)]
    )
    NCH = len(chunks)

    BANK = 512  # f32 elements per PSUM bank
    N_BANKS = FEAT // BANK

    thr_row = nc.alloc_sbuf_tensor("thr_row", [1, FEAT], mybir.dt.float32)
    ones = nc.alloc_sbuf_tensor("ones", [1, P], mybir.dt.float32)
    # Thresholds replicated across partitions live in PSUM for the whole
    # kernel (nothing else needs PSUM); tensor_tensor reads in1 from there.
    thr_ps = nc.alloc_psum_tensor("thr_ps", [P, FEAT], mybir.dt.float32)
    tiles = [
        nc.alloc_sbuf_tensor(f"t{i}", [P, w], mybir.dt.float32)
        for i, (_, _, w) in enumerate(chunks)
    ]

    with (
        nc.Block() as block,
        nc.semaphore("bc_sem") as bc_sem,
        nc.semaphore("ones_sem") as ones_sem,
        nc.semaphore("mm_sem") as mm_sem,
        nc.semaphore("cv_sem") as cv_sem,
    ):
        # One sem per in-flight DMA: completion increments from concurrent
        # DMAs on one shared sem would make intermediate wait values
        # ill-defined (the 16 per-SDMA-engine incs of different DMAs
        # interleave). Stores reuse the load sems (16 -> 32) — strictly
        # ordered through the compute, and ACT re-waits >=16 first.
        ld_sems = [nc.alloc_semaphore(f"ld{i}") for i in range(NCH)]

        @block.sync
        def _(sync: bass.BassEngine):
            # 16 KB threshold row first — lands almost immediately.
            sync.dma_start(out=thr_row.ap(), in_=thr.ap().unsqueeze(0)).then_inc(
                bc_sem, 16
            )
            for i, (r, f0, w) in enumerate(chunks):
                sync.dma_start(
                    out=tiles[i].ap(), in_=x3[:, r, bass.ds(f0, w)]
                ).then_inc(ld_sems[i], 16)

        @block.gpsimd
        def _(gpsimd: bass.BassEngine):
            gpsimd.memset(ones.ap(), 1.0).then_inc(ones_sem, 1)

        @block.tensor
        def _(tensor: bass.BassEngine):
            # Replicate thr across partitions off the DMA fabric:
            # ones[1,128].T @ thr_row[1,512] -> PSUM bank [128,512].
            tensor.wait_ge(ones_sem, 1)
            tensor.wait_ge(bc_sem, 16)
            for j in range(N_BANKS):
                tensor.matmul(
                    thr_ps.ap()[:, bass.ds(j * BANK, BANK)],
                    ones.ap(),
                    thr_row.ap()[:, bass.ds(j * BANK, BANK)],
                    start=True,
                    stop=True,
                ).then_inc(mm_sem, 1)

        @block.vector
        def _(vector: bass.BassEngine):
            for i, (r, f0, w) in enumerate(chunks):
                vector.wait_ge(mm_sem, (f0 + w) // BANK)  # banks covering chunk
                vector.wait_ge(ld_sems[i], 16)
                vector.tensor_tensor(
                    tiles[i].ap(),
                    tiles[i].ap(),
                    thr_ps.ap()[:, bass.ds(f0, w)],
                    mybir.AluOpType.is_ge,
                ).then_inc(cv_sem, 1)

        @block.scalar
        def _(scalar: bass.BassEngine):
            for i, (r, f0, w) in enumerate(chunks):
                scalar.wait_ge(cv_sem, i + 1)
                scalar.wait_ge(ld_sems[i], 16)
                scalar.dma_start(
                    out=out3[:, r, bass.ds(f0, w)], in_=tiles[i].ap()
                ).then_inc(ld_sems[i], 16)
            for i in range(NCH):
                scalar.wait_ge(ld_sems[i], 32)
            # Observe the remaining sems' final values so the post-barrier
            # clears can't race an in-flight update.
            scalar.wait_ge(bc_sem, 16)
            scalar.wait_ge(ones_sem, 1)
            scalar.wait_ge(mm_sem, N_BANKS)

    # Everything has quiesced (the Block exit above emits a full drain +
    # all-engine barrier): zero the sems so a re-execution of the same
    # loaded NEFF starts from a clean state.
    for s in [bc_sem, ones_sem, mm_sem, cv_sem, *ld_sems]:
        nc.scalar.sem_clear(s)

    return nc


def _run(inputs, medians, **spmd_kwargs):
    global _module
    if _module is None:
        _module = _build_module()
    inputs = np.ascontiguousarray(np.asarray(inputs, dtype=np.float32))
    medians = np.asarray(medians, dtype=np.float32)
    thr = np.where(medians > 0.0, medians, BIG).astype(np.float32)
    in_maps = [
        {"inputs": inputs[i * SHARD:(i + 1) * SHARD], "thresholds": thr}
        for i in range(N_CORES)
    ]
    res = run_bass_kernel_spmd(_module, in_maps, list(range(N_CORES)), **spmd_kwargs)
    full = np.concatenate([res.results[i]["output"] for i in range(N_CORES)], axis=0)
    return full, res


def kernel(inputs, medians):
    full, _ = _run(inputs, medians)
    return full


# revision 15
# speedup vs baseline: 1.0162x; 1.0162x over previous
"""Trainium2 Bass kernel for nn_BinarizeLayer (histogram_binning).

out[b, f] = 1.0 if (medians[f] > 0) and (inputs[b, f] >= medians[f]) else 0.0

Sharding: pure data-parallel over batch — each of the 8 cores processes a
[1024, 4096] contiguous row shard; the 16 KB medians vector is replicated.

The (median > 0) gate is folded into a per-feature threshold on the host
(thr[f] = medians[f] if medians[f] > 0 else FLT_MAX, a 4096-element
np.where) so the device hot loop is one DVE is_ge compare per element:
inputs are finite floats far below FLT_MAX, so x >= FLT_MAX is never true.

Raw Bass (no Tile): this walrus rejects any instruction carrying more
than one sync-wait, which Tile's generated schedules (and its kernel-tail
drain) violate. With explicit semaphores every wait is its own
single-wait instruction. Pipeline: SP streams the 8 input-chunk loads,
DVE compares each chunk in place as its load lands, ACT streams the
stores behind the compares — loads and stores ride separate HWDGE rings.
"""

import numpy as np

import concourse.bass as bass
import concourse.mybir as mybir
from concourse.bass_utils import run_bass_kernel_spmd

N_CORES = 8
BATCH, FEAT = 8192, 4096
SHARD = BATCH // N_CORES  # 1024 rows per core
P = 128                   # SBUF partitions
ROWG = SHARD // P         # 8 row-groups; DRAM row = p * ROWG + r
BIG = np.float32(3.4e38)  # gate-closed sentinel; x >= BIG never true for inputs

_module = None


def _build_module():
    nc = bass.Bass()
    x = nc.declare_dram_parameter("inputs", [SHARD, FEAT], mybir.dt.float32, isOutput=False)
    thr = nc.declare_dram_parameter("thresholds", [FEAT], mybir.dt.float32, isOutput=False)
    out = nc.declare_dram_parameter("output", [SHARD, FEAT], mybir.dt.float32, isOutput=True)

    # Partition p owns contiguous DRAM rows [p*ROWG, (p+1)*ROWG): each
    # partition's slice of chunk r is one contiguous 16 KB run.
    x3 = x.ap().rearrange("(p r) f -> p r f", p=P)
    out3 = out.ap().rearrange("(p r) f -> p r f", p=P)

    # Chunks: (row-group r, feature offset, width). Small first chunks
    # shorten the ramp (first compute starts sooner); small last chunks
    # shorten the load→compute→store→receipt tail.
    H = FEAT // 2
    chunks = (
        [(0, 0, H), (0, H, H)]
        + [(r, 0, FEAT) for r in range(1, ROWG - 1)]
        + [(ROWG - 1, 0, H), (ROWG - 1, H, H)]
    )
    NCH = len(chunks)

    BANK = 512  # f32 elements per PSUM bank
    N_BANKS = FEAT // BANK

    thr_row = nc.alloc_sbuf_tensor("thr_row", [1, FEAT], mybir.dt.float32)
    ones = nc.alloc_sbuf_tensor("ones", [1, P], mybir.dt.float32)
    # Thresholds replicated across partitions live in PSUM for the whole
    # kernel (nothing else needs PSUM); tensor_tensor reads in1 from there.
    thr_ps = nc.alloc_psum_tensor("thr_ps", [P, FEAT], mybir.dt.float32)
    tiles = [
        nc.alloc_sbuf_tensor(f"t{i}", [P, w], mybir.dt.float32)
        for i, (_, _, w) in enumerate(chunks)
    ]

    with (
        nc.Block() as block,
        nc.semaphore("bc_sem") as bc_sem,
        nc.semaphore("ones_sem") as ones_sem,
        nc.semaphore("mm_sem") as mm_sem,
        nc.semaphore("cv_sem") as cv_sem,
    ):
        # One sem per in-flight DMA: completion increments from concurrent
        # DMAs on one shared sem would make intermediate wait values
        # ill-defined (the 16 per-SDMA-engine incs of different DMAs
        # interleave). Stores reuse the load sems (16 -> 32) — strictly
        # ordered through the compute, and ACT re-waits >=16 first.
        ld_sems = [nc.alloc_semaphore(f"ld{i}") for i in range(NCH)]

        @block.sync
        def _(sync: bass.BassEngine):
            # 16 KB threshold row first — lands almost immediately.
            sync.dma_start(out=thr_row.ap(), in_=thr.ap().unsqueeze(0)).then_inc(
                bc_sem, 16
            )
            for i, (r, f0, w) in enumerate(chunks):
                sync.dma_start(
                    out=tiles[i].ap(), in_=x3[:, r, bass.ds(f0, w)]
                ).then_inc(ld_sems[i], 16)

        @block.gpsimd
        def _(gpsimd: bass.BassEngine):
            gpsimd.memset(ones.ap(), 1.0).then_inc(ones_sem, 1)

        @block.tensor
        def _(tensor: bass.BassEngine):
            # Replicate thr across partitions off the DMA fabric:
            # ones[1,128].T @ thr_row[1,512] -> PSUM bank [128,512].
            tensor.wait_ge(ones_sem, 1)
            tensor.wait_ge(bc_sem, 16)
            for j in range(N_BANKS):
                tensor.matmul(
                    thr_ps.ap()[:, bass.ds(j * BANK, BANK)],
                    ones.ap(),
                    thr_row.ap()[:, bass.ds(j * BANK, BANK)],
                    start=True,
                    stop=True,
                ).then_inc(mm_sem, 1)

        @block.vector
        def _(vector: bass.BassEngine):
            for i, (r, f0, w) in enumerate(chunks):
                vector.wait_ge(mm_sem, (f0 + w) // BANK)  # banks covering chunk
                vector.wait_ge(ld_sems[i], 16)
                vector.tensor_tensor(
                    tiles[i].ap(),
                    tiles[i].ap(),
                    thr_ps.ap()[:, bass.ds(f0, w)],
                    mybir.AluOpType.is_ge,
                ).then_inc(cv_sem, 1)

        @block.scalar
        def _(scalar: bass.BassEngine):
            for i, (r, f0, w) in enumerate(chunks):
                scalar.wait_ge(cv_sem, i + 1)
                scalar.wait_ge(ld_sems[i], 16)
                scalar.dma_start(
                    out=out3[:, r, bass.ds(f0, w)], in_=tiles[i].ap()
                ).then_inc(ld_sems[i], 16)
            for i in range(NCH):
                scalar.wait_ge(ld_sems[i], 32)
            # Observe the remaining sems' final values so the post-barrier
            # clears can't race an in-flight update.
            scalar.wait_ge(bc_sem, 16)
            scalar.wait_ge(ones_sem, 1)
            scalar.wait_ge(mm_sem, N_BANKS)

    # Everything has quiesced (the Block exit above emits a full drain +
    # all-engine barrier): zero the sems so a re-execution of the same
    # loaded NEFF starts from a clean state.
    for s in [bc_sem, ones_sem, mm_sem, cv_sem, *ld_sems]:
        nc.scalar.sem_clear(s)

    return nc


def _run(inputs, medians, **spmd_kwargs):
    global _module
    if _module is None:
        _module = _build_module()
    inputs = np.ascontiguousarray(np.asarray(inputs, dtype=np.float32))
    medians = np.asarray(medians, dtype=np.float32)
    thr = np.where(medians > 0.0, medians, BIG).astype(np.float32)
    in_maps = [
        {"inputs": inputs[i * SHARD:(i + 1) * SHARD], "thresholds": thr}
        for i in range(N_CORES)
    ]
    res = run_bass_kernel_spmd(_module, in_maps, list(range(N_CORES)), **spmd_kwargs)
    full = np.concatenate([res.results[i]["output"] for i in range(N_CORES)], axis=0)
    return full, res


def kernel(inputs, medians):
    full, _ = _run(inputs, medians)
    return full


# revision 18
# speedup vs baseline: 1.1551x; 1.1367x over previous
"""Trainium2 Bass kernel for nn_BinarizeLayer (histogram_binning).

out[b, f] = 1.0 if (medians[f] > 0) and (inputs[b, f] >= medians[f]) else 0.0

Sharding: pure data-parallel over batch — each of the 8 cores processes a
[1024, 4096] contiguous row shard; the 16 KB medians vector is replicated.

The (median > 0) gate is folded into a per-feature threshold on the host
(thr[f] = medians[f] if medians[f] > 0 else FLT_MAX, a 4096-element
np.where) so the device hot loop is one DVE is_ge compare per element:
inputs are finite floats far below FLT_MAX, so x >= FLT_MAX is never true.

Raw Bass (no Tile): this walrus rejects any instruction carrying more
than one sync-wait, which Tile's generated schedules (and its kernel-tail
drain) violate. With explicit semaphores every wait is its own
single-wait instruction. Pipeline: SP streams the 8 input-chunk loads,
DVE compares each chunk in place as its load lands, ACT streams the
stores behind the compares — loads and stores ride separate HWDGE rings.
"""

import numpy as np

import concourse.bass as bass
import concourse.mybir as mybir
from concourse.bass_utils import run_bass_kernel_spmd

N_CORES = 8
BATCH, FEAT = 8192, 4096
SHARD = BATCH // N_CORES  # 1024 rows per core
P = 128                   # SBUF partitions
ROWG = SHARD // P         # 8 row-groups; DRAM row = p * ROWG + r
BIG = np.float32(3.4e38)  # gate-closed sentinel; x >= BIG never true for inputs

_module = None


def _build_module():
    nc = bass.Bass()
    x = nc.declare_dram_parameter("inputs", [SHARD, FEAT], mybir.dt.float32, isOutput=False)
    thr = nc.declare_dram_parameter("thresholds", [FEAT], mybir.dt.float32, isOutput=False)
    out = nc.declare_dram_parameter("output", [SHARD, FEAT], mybir.dt.float32, isOutput=True)

    # Partition p owns contiguous DRAM rows [p*ROWG, (p+1)*ROWG): each
    # partition's slice of chunk r is one contiguous 16 KB run.
    x3 = x.ap().rearrange("(p r) f -> p r f", p=P)
    out3 = out.ap().rearrange("(p r) f -> p r f", p=P)

    # Chunks: (row-group r, feature offset, width). Small first chunks
    # shorten the ramp (first compute starts sooner); small last chunks
    # shorten the load→compute→store→receipt tail.
    H = FEAT // 2
    chunks = (
        [(0, 0, H), (0, H, H)]
        + [(r, 0, FEAT) for r in range(1, ROWG - 1)]
        + [(ROWG - 1, 0, H), (ROWG - 1, H, H)]
    )
    NCH = len(chunks)

    BANK = 512  # f32 elements per PSUM bank
    N_BANKS = FEAT // BANK

    thr_row = nc.alloc_sbuf_tensor("thr_row", [1, FEAT], mybir.dt.float32)
    ones = nc.alloc_sbuf_tensor("ones", [1, P], mybir.dt.float32)
    # Thresholds replicated across partitions live in PSUM for the whole
    # kernel (nothing else needs PSUM); tensor_tensor reads in1 from there.
    thr_ps = nc.alloc_psum_tensor("thr_ps", [P, FEAT], mybir.dt.float32)
    tiles = [
        nc.alloc_sbuf_tensor(f"t{i}", [P, w], mybir.dt.float32)
        for i, (_, _, w) in enumerate(chunks)
    ]

    with (
        nc.Block(no_gpsimd_drain=True) as block,
        nc.semaphore("bc_sem") as bc_sem,
        nc.semaphore("ones_sem") as ones_sem,
        nc.semaphore("mm_sem") as mm_sem,
        nc.semaphore("cv_sem") as cv_sem,
    ):
        # One sem per in-flight DMA: completion increments from concurrent
        # DMAs on one shared sem would make intermediate wait values
        # ill-defined (the 16 per-SDMA-engine incs of different DMAs
        # interleave). Stores reuse the load sems (16 -> 32) — strictly
        # ordered through the compute, and ACT re-waits >=16 first.
        ld_sems = [nc.alloc_semaphore(f"ld{i}") for i in range(NCH)]

        @block.sync
        def _(sync: bass.BassEngine):
            # 16 KB threshold row first — lands almost immediately.
            sync.dma_start(out=thr_row.ap(), in_=thr.ap().unsqueeze(0)).then_inc(
                bc_sem, 16
            )
            for i, (r, f0, w) in enumerate(chunks):
                sync.dma_start(
                    out=tiles[i].ap(), in_=x3[:, r, bass.ds(f0, w)]
                ).then_inc(ld_sems[i], 16)

        @block.tensor
        def _(tensor: bass.BassEngine):
            # Replicate thr across partitions off the DMA fabric:
            # ones[1,128].T @ thr_row[1,512] -> PSUM bank [128,512].
            tensor.wait_ge(ones_sem, 1)
            tensor.wait_ge(bc_sem, 16)
            for j in range(N_BANKS):
                tensor.matmul(
                    thr_ps.ap()[:, bass.ds(j * BANK, BANK)],
                    ones.ap(),
                    thr_row.ap()[:, bass.ds(j * BANK, BANK)],
                    start=True,
                    stop=True,
                ).then_inc(mm_sem, 1)

        @block.vector
        def _(vector: bass.BassEngine):
            vector.memset(ones.ap(), 1.0).then_inc(ones_sem, 1)
            for i, (r, f0, w) in enumerate(chunks):
                vector.wait_ge(mm_sem, (f0 + w) // BANK)  # banks covering chunk
                vector.wait_ge(ld_sems[i], 16)
                vector.tensor_tensor(
                    tiles[i].ap(),
                    tiles[i].ap(),
                    thr_ps.ap()[:, bass.ds(f0, w)],
                    mybir.AluOpType.is_ge,
                ).then_inc(cv_sem, 1)

        @block.scalar
        def _(scalar: bass.BassEngine):
            for i, (r, f0, w) in enumerate(chunks):
                scalar.wait_ge(cv_sem, i + 1)
                scalar.wait_ge(ld_sems[i], 16)
                scalar.dma_start(
                    out=out3[:, r, bass.ds(f0, w)], in_=tiles[i].ap()
                ).then_inc(ld_sems[i], 16)
            for i in range(NCH):
                scalar.wait_ge(ld_sems[i], 32)
            # Observe the remaining sems' final values so the post-barrier
            # clears can't race an in-flight update.
            scalar.wait_ge(bc_sem, 16)
            scalar.wait_ge(ones_sem, 1)
            scalar.wait_ge(mm_sem, N_BANKS)

    # Everything has quiesced (the Block exit above emits a full drain +
    # all-engine barrier): zero the sems so a re-execution of the same
    # loaded NEFF starts from a clean state.
    for s in [bc_sem, ones_sem, mm_sem, cv_sem, *ld_sems]:
        nc.scalar.sem_clear(s)

    return nc


def _run(inputs, medians, **spmd_kwargs):
    global _module
    if _module is None:
        _module = _build_module()
    inputs = np.ascontiguousarray(np.asarray(inputs, dtype=np.float32))
    medians = np.asarray(medians, dtype=np.float32)
    thr = np.where(medians > 0.0, medians, BIG).astype(np.float32)
    in_maps = [
        {"inputs": inputs[i * SHARD:(i + 1) * SHARD], "thresholds": thr}
        for i in range(N_CORES)
    ]
    res = run_bass_kernel_spmd(_module, in_maps, list(range(N_CORES)), **spmd_kwargs)
    full = np.concatenate([res.results[i]["output"] for i in range(N_CORES)], axis=0)
    return full, res


def kernel(inputs, medians):
    full, _ = _run(inputs, medians)
    return full
